# revision 4
# baseline (speedup 1.0000x reference)
"""Multi-Query Attention kernel for 8x TRN2 NeuronCores (Bass/Tile).

Problem: x[B=2, L=2048, D=2048], Wq[2048,2048], Wk/Wv[128,2048] (MQA: one
shared K/V head), 16 query heads of dim 128.

Sharding: core c in [0,8): batch b = c//4, head-group g = c%4 (4 heads,
i.e. q-channels [512g, 512g+512)). K/V replicated per core.

v2 schedule ("KV-first + merged supersteps"): the baseline ran projections
then attention as two phases; the attention phase was ACT(exp)-paced, so the
PE idled ~0.4us at every pass boundary and each stall re-triggered the PE
p-state ramp (matmuls stretched 216 -> up to 607ns).  This version keeps the
PE continuously busy:

  1. x streams into SBUF ONCE (64 [128,512] pieces, issue spread over the
     vector/gpsimd/scalar DGEs so the SP queue stays short) and stays
     resident (64KB/partition).
  2. K/V projections for all of L run first (PE-bound, ACT nearly idle),
     then Q l-tile 0 and the 16 V transposes.
  3. Supersteps t=0..3: attention passes (lq=t, head-pairs 0,1) emit their
     scores/AV matmuls interleaved with the NEXT Q l-tile's projection
     matmuls as PE filler.  Per 128-key block the PE does 6 matmuls
     (~1.3us) vs ACT's one 1.0us exp, so the exp pipeline (and the DVE
     denominator adds) hide completely under PE work.
  4. A pass's softmax tail (ones-matmul partition-reduce of the DVE-summed
     denominator -> fast reciprocal -> normalize multiply -> output DMA) is
     emitted inside the NEXT pass's boundary, after the next scores have
     been issued, so the in-order PE never waits on the exp->add chain.
     The AV accumulator is drained PSUM->SBUF by the ACT right after the
     last AV matmul, freeing its 2 PSUM banks for the next pass (PSUM:
     2 Q-projection banks + 2x2 scores banks + 2 AV banks = 8 exactly).

Precision (identical math to the HW-verified baseline, rel_err ~2.0e-3 vs
the 2e-2 budget): x/W stream fp16, projections fp16 x fp16 -> fp32 PSUM,
Q/K kept fp16, exp output and V bf16 (DVE accumulates the denominator at
its 2x 16-bit rate), everything normalized in fp32.
"""

from contextlib import ExitStack

import numpy as np

import concourse.bass as bass
import concourse.tile as tile
from concourse import bacc, masks, mybir
from concourse.bass_utils import run_bass_kernel_spmd

F32 = mybir.dt.float32
BF16 = mybir.dt.bfloat16
F16 = mybir.dt.float16
AF = mybir.ActivationFunctionType

B = 2
L = 2048
D = 2048  # d_model (contraction dim of projections)
HD = 128  # head dim
NH = 4  # heads per core
QC = NH * HD  # q-channels per core = 512
DC = D // 128  # d-model chunks of 128 = 16
NLT = 4  # l tiles of 512
LKT = L // 128  # lk blocks of 128 = 16
N_CORES = 8
SCALE = 1.0 / float(np.sqrt(HD))


def build_kernel(ctx: ExitStack, tc: tile.TileContext, xT, wkvT, wqT, bq, bk, bv, outT):
    nc = tc.nc

    persist = ctx.enter_context(tc.tile_pool(name="persist", bufs=1))
    x_sb = [persist.tile([128, L], F16, tag=f"x{k}", name=f"x{k}") for k in range(DC)]
    wkv = [persist.tile([128, 2 * HD], F16, tag=f"wkv{k}", name=f"wkv{k}") for k in range(DC)]
    wq = [persist.tile([128, QC], F16, tag=f"wq{k}", name=f"wq{k}") for k in range(DC)]
    qT = [persist.tile([128, L], F16, tag=f"qT{h}", name=f"qT{h}") for h in range(NH)]  # [d, l]
    kT = persist.tile([128, L], F16, tag="kT", name="kT")  # [d, l]
    vT = [persist.tile([128, 512], BF16, tag=f"vT{t}", name=f"vT{t}") for t in range(NLT)]
    vN = persist.tile([128, L], BF16, tag="vN", name="vN")  # block j: [:, 128j:+128] = V[128j:+128, :]
    ones_bf = persist.tile([128, 128], BF16, tag="ones_bf", name="ones_bf")
    ident = persist.tile([128, 128], BF16, tag="ident", name="ident")
    bq_sb = persist.tile([128, NH], F32, tag="bq", name="bq")
    bk_sb = persist.tile([128, 1], F32, tag="bk", name="bk")
    bv_sb = persist.tile([128, 1], F32, tag="bv", name="bv")

    nc.vector.memset(ones_bf[:], 1.0)
    masks.make_identity(nc, ident[:])
    nc.scalar.dma_start(out=bq_sb[:], in_=bq)
    nc.scalar.dma_start(out=bk_sb[:], in_=bk)
    nc.scalar.dma_start(out=bv_sb[:], in_=bv)

    # ---------------- DMA plan ----------------
    # wkv chunks first on the SP queue (KV phase gates on them), then wq.
    # x pieces round-robin over the vector/gpsimd/scalar DGEs: 64 issues
    # would swamp the ~1.6 issues/us SP queue, and these engines are idle
    # during the first half of the kernel anyway.
    for k in range(DC):
        nc.sync.dma_start(out=wkv[k][:], in_=wkvT[k * 128:(k + 1) * 128, :])
    engs = [nc.gpsimd, nc.scalar]
    i = 0
    for lt in range(NLT):
        for k in range(DC):
            engs[i % 2].dma_start(
                out=x_sb[k][:, lt * 512:(lt + 1) * 512],
                in_=xT[k * 128:(k + 1) * 128, lt * 512:(lt + 1) * 512],
            )
            i += 1
    for k in range(DC):
        nc.sync.dma_start(out=wq[k][:], in_=wqT[k * 128:(k + 1) * 128, :])

    # ---------------- PE warmup ----------------
    # The PE runs at 0.65/1.2 GHz until it has been continuously busy ~3us;
    # ramp it on throwaway matmuls while the first wkv/x DMAs land.
    with tc.tile_pool(name="warm", bufs=1, space="PSUM") as wp:
        pw = wp.tile([128, 128], F32, tag="warm", name="pw")
        for _ in range(30):
            nc.tensor.matmul(pw[:], lhsT=ones_bf[:], rhs=ident[:], start=True, stop=True)

    # ---------------- KV phase: kT/vT for all of L ----------------
    with tc.tile_pool(name="kvp", bufs=2, space="PSUM") as kvp:
        for lt in range(NLT):
            ls = slice(lt * 512, (lt + 1) * 512)
            psk = kvp.tile([128, 512], F32, tag="psk", name="psk")
            psv = kvp.tile([128, 512], F32, tag="psv", name="psv")
            for k in range(DC):
                st = k == 0
                sp = k == DC - 1
                nc.tensor.matmul(psk[:], lhsT=wkv[k][:, 0:HD], rhs=x_sb[k][:, ls], start=st, stop=sp)
                nc.tensor.matmul(psv[:], lhsT=wkv[k][:, HD:], rhs=x_sb[k][:, ls], start=st, stop=sp)
            nc.scalar.activation(kT[:, ls], psk[:], AF.Identity, bias=bk_sb[:, 0:1])
            nc.scalar.activation(vT[lt][:], psv[:], AF.Identity, bias=bv_sb[:, 0:1])

    # ---------------- Q projections (as superstep filler) ----------------
    qp = ctx.enter_context(tc.tile_pool(name="qp", bufs=1, space="PSUM"))

    def q_sweep_items(t, pair):
        """Generator of emission thunks: 32 matmuls (heads 2*pair, 2*pair+1
        over 16 d-chunks) + 2 ACT drains. Lazily allocates its 2 PSUM tiles
        on first next()."""
        ls = slice(t * 512, (t + 1) * 512)
        ps = [qp.tile([128, 512], F32, tag=f"psq{j}", name=f"psq{j}") for j in (0, 1)]
        for k in range(DC):
            st = k == 0
            sp = k == DC - 1
            for j in (0, 1):
                h = 2 * pair + j
                yield lambda k=k, j=j, h=h, st=st, sp=sp: nc.tensor.matmul(
                    ps[j][:],
                    lhsT=wq[k][:, h * 128:(h + 1) * 128],
                    rhs=x_sb[k][:, ls],
                    start=st,
                    stop=sp,
                )
        for j in (0, 1):
            h = 2 * pair + j
            yield lambda j=j, h=h: nc.scalar.activation(
                qT[h][:, ls], ps[j][:], AF.Identity, bias=bq_sb[:, h:h + 1]
            )

    def run_all(gen):
        for th in gen:
            th()

    # Q l-tile 0 runs before the first superstep; the V transposes slot
    # between its two sweeps (their tpg PSUM bank is free again before the
    # attention pools open).
    run_all(q_sweep_items(0, 0))
    with tc.tile_pool(name="tpg", bufs=2, space="PSUM") as tpg:
        for j in range(LKT):
            pt = tpg.tile([128, 128], BF16, tag="tp", name="tp")
            nc.tensor.transpose(pt[:], vT[j // 4][:, (j % 4) * 128:(j % 4 + 1) * 128], ident[:])
            nc.scalar.activation(vN[:, j * 128:(j + 1) * 128], pt[:], AF.Identity)
    run_all(q_sweep_items(0, 1))

    # ---------------- attention supersteps ----------------
    sps = ctx.enter_context(tc.tile_pool(name="sps", bufs=2, space="PSUM"))  # 2 x [128,1024] = 4 banks
    avp = ctx.enter_context(tc.tile_pool(name="avp", bufs=1, space="PSUM"))  # [128,1024] = 2 banks
    attp = ctx.enter_context(tc.tile_pool(name="att", bufs=6))
    accp = ctx.enter_context(tc.tile_pool(name="acc", bufs=2))
    avsbp = ctx.enter_context(tc.tile_pool(name="avsb", bufs=2))
    finp = ctx.enter_context(tc.tile_pool(name="fin", bufs=3))

    tail = {"pending": None}

    def emit_tail(info, split=2):
        """Softmax tail of a finished pass: partition-reduce+replicate the
        DVE-summed denominator with two 128-wide ones-matmuls (into a
        rotating scores PSUM slot), fast-reciprocal, normalize the
        ACT-drained AV sum, DMA out."""
        acc, av_sb, t, hp = info
        psR = sps.tile([128, 1024], F32, tag="sps", name="psR")
        for j in (0, 1):
            nc.tensor.matmul(
                psR[:, j * 512:(j + 1) * 512],
                lhsT=ones_bf[:],
                rhs=acc[:, j * 512:(j + 1) * 512],
                start=True,
                stop=True,
            )
        rinv = finp.tile([128, 1024], F32, tag="rinv", name="rinv")
        nc.vector.reciprocal_approx_fast(rinv[:], psR[:])
        ot = finp.tile([128, 1024], F32, tag="ot", name="ot")
        nc.vector.tensor_mul(ot[:], av_sb[:], rinv[:])
        n = 512 // split
        for j in (0, 1):
            h = 2 * hp + j
            for s in range(split):
                cs = slice(t * 512 + s * n, t * 512 + (s + 1) * n)
                nc.sync.dma_start(out=outT[h * 128:(h + 1) * 128, cs], in_=ot[:, j * 512 + s * n:j * 512 + (s + 1) * n])

    def emit_pass(t, hp, fill):
        qs = slice(t * 512, (t + 1) * 512)
        acc = accp.tile([128, 1024], BF16, tag="acc", name="acc")
        psA = avp.tile([128, 1024], F32, tag="av", name="av")
        at_of = {}

        def F(n=1):
            for _ in range(n):
                th = next(fill, None)
                if th is not None:
                    th()

        def sc(lk):
            ss = sps.tile([128, 1024], F32, tag="sps", name="sps")
            for j in (0, 1):
                nc.tensor.matmul(
                    ss[:, j * 512:(j + 1) * 512],
                    lhsT=kT[:, lk * 128:(lk + 1) * 128],
                    rhs=qT[2 * hp + j][:, qs],
                    start=True,
                    stop=True,
                )
            at = attp.tile([128, 1024], BF16, tag="att", name="att")
            nc.scalar.activation(at[:], ss[:], AF.Exp, scale=SCALE)
            # softmax denominator partials on the DVE (2x bf16 rate)
            if lk == 0:
                nc.vector.tensor_copy(acc[:], at[:])
            else:
                nc.vector.tensor_add(acc[:], acc[:], at[:])
            at_of[lk] = at

        def av(lk):
            for j in (0, 1):
                nc.tensor.matmul(
                    psA[:, j * 512:(j + 1) * 512],
                    lhsT=vN[:, lk * 128:(lk + 1) * 128],
                    rhs=at_of[lk][:, j * 512:(j + 1) * 512],
                    start=lk == 0,
                    stop=lk == LKT - 1,
                )

        # boundary: scores for blocks 0-1, fillers to cover the previous
        # pass's exp->add->psR chain, then that pass's tail
        sc(0)
        F(2)
        sc(1)
        F(4)
        if tail["pending"] is not None:
            emit_tail(tail["pending"])
            tail["pending"] = None
        F(4)
        # steady state: AV trails scores by 2 blocks, ~2 fillers per block
        for lk in range(2, LKT):
            sc(lk)
            F(1)
            av(lk - 2)
            if lk % 4 == 1:
                F(1)
        av(LKT - 2)
        F(1)
        av(LKT - 1)
        # drain the AV accumulator to SBUF on the ACT (frees its PSUM banks
        # for the next pass; the normalize multiply reads the SBUF copy)
        av_sb = avsbp.tile([128, 1024], F32, tag="avsb", name="avsb")
        nc.scalar.activation(av_sb[:], psA[:], AF.Identity)
        tail["pending"] = (acc, av_sb, t, hp)

    for t in range(NLT):
        if t + 1 < NLT:
            import itertools
            fill = itertools.chain(q_sweep_items(t + 1, 0), q_sweep_items(t + 1, 1))
        else:
            fill = iter(())
        emit_pass(t, 0, fill)
        emit_pass(t, 1, fill)
        run_all(fill)  # leftover fillers
    emit_tail(tail["pending"], split=4)
    tail["pending"] = None


_NC_CACHE = None


def build_nc():
    global _NC_CACHE
    if _NC_CACHE is not None:
        return _NC_CACHE
    nc = bacc.Bacc("TRN2", target_bir_lowering=False, debug=False)
    xT = nc.dram_tensor("xT", [D, L], F16, kind="ExternalInput").ap()
    wkvT = nc.dram_tensor("wkvT", [D, 2 * HD], F16, kind="ExternalInput").ap()
    wqT = nc.dram_tensor("wqT", [D, QC], F16, kind="ExternalInput").ap()
    bq = nc.dram_tensor("bq", [128, NH], F32, kind="ExternalInput").ap()
    bk = nc.dram_tensor("bk", [128, 1], F32, kind="ExternalInput").ap()
    bv = nc.dram_tensor("bv", [128, 1], F32, kind="ExternalInput").ap()
    outT = nc.dram_tensor("outT", [QC, L], F32, kind="ExternalOutput").ap()
    with tile.TileContext(nc) as tc, ExitStack() as ctx:
        build_kernel(ctx, tc, xT, wkvT, wqT, bq, bk, bv, outT)
    nc.compile()
    _NC_CACHE = nc
    return nc


def make_in_maps(x, Wq_w, Wq_b, Wk_w, Wk_b, Wv_w, Wv_b):
    """Host-side sharding/relayout. Returns one input map per core."""
    x = np.asarray(x, dtype=np.float32)
    Wq_w = np.asarray(Wq_w, dtype=np.float32)
    Wq_b = np.asarray(Wq_b, dtype=np.float32)
    Wk_w = np.asarray(Wk_w, dtype=np.float32)
    Wk_b = np.asarray(Wk_b, dtype=np.float32)
    Wv_w = np.asarray(Wv_w, dtype=np.float32)
    Wv_b = np.asarray(Wv_b, dtype=np.float32)

    xTs = [np.ascontiguousarray(x[b].T).astype(np.float16) for b in range(B)]
    wkvT = np.ascontiguousarray(
        np.concatenate([Wk_w.T, Wv_w.T], axis=1)
    ).astype(np.float16)  # [D, 256]
    bk = np.ascontiguousarray(Wk_b.reshape(128, 1))
    bv = np.ascontiguousarray(Wv_b.reshape(128, 1))
    in_maps = []
    for c in range(N_CORES):
        b, g = divmod(c, B * 2)  # b = c // 4, g = c % 4
        wqT_g = np.ascontiguousarray(Wq_w[g * QC:(g + 1) * QC, :].T).astype(np.float16)
        bq_g = np.ascontiguousarray(Wq_b[g * QC:(g + 1) * QC].reshape(NH, 128).T)
        in_maps.append(
            {
                "xT": xTs[b],
                "wkvT": wkvT,
                "wqT": wqT_g,
                "bq": bq_g,
                "bk": bk,
                "bv": bv,
            }
        )
    return in_maps


def assemble_output(results):
    out = np.empty((B, L, D), dtype=np.float32)
    for c in range(N_CORES):
        b, g = divmod(c, B * 2)
        out[b, :, g * QC:(g + 1) * QC] = results[c]["outT"].T
    return out


def kernel(**inputs) -> np.ndarray:
    nc = build_nc()
    in_maps = make_in_maps(**inputs)
    res = run_bass_kernel_spmd(nc, in_maps, core_ids=list(range(N_CORES)))
    return assemble_output(res.results)


# revision 7
# speedup vs baseline: 1.0528x; 1.0528x over previous
"""Multi-Query Attention kernel for 8x TRN2 NeuronCores (Bass/Tile).

Problem: x[B=2, L=2048, D=2048], Wq[2048,2048], Wk/Wv[128,2048] (MQA: one
shared K/V head), 16 query heads of dim 128.

Sharding: core c in [0,8): batch b = c//4, head-group g = c%4 (4 heads,
i.e. q-channels [512g, 512g+512)). K/V replicated per core.

v2 schedule ("KV-first + merged supersteps"): the baseline ran projections
then attention as two phases; the attention phase was ACT(exp)-paced, so the
PE idled ~0.4us at every pass boundary and each stall re-triggered the PE
p-state ramp (matmuls stretched 216 -> up to 607ns).  This version keeps the
PE continuously busy:

  1. x streams into SBUF ONCE (64 [128,512] pieces, issue spread over the
     vector/gpsimd/scalar DGEs so the SP queue stays short) and stays
     resident (64KB/partition).
  2. K/V projections for all of L run first (PE-bound, ACT nearly idle),
     then Q l-tile 0 and the 16 V transposes.
  3. Supersteps t=0..3: attention passes (lq=t, head-pairs 0,1) emit their
     scores/AV matmuls interleaved with the NEXT Q l-tile's projection
     matmuls as PE filler.  Per 128-key block the PE does 6 matmuls
     (~1.3us) vs ACT's one 1.0us exp, so the exp pipeline (and the DVE
     denominator adds) hide completely under PE work.
  4. A pass's softmax tail (ones-matmul partition-reduce of the DVE-summed
     denominator -> fast reciprocal -> normalize multiply -> output DMA) is
     emitted inside the NEXT pass's boundary, after the next scores have
     been issued, so the in-order PE never waits on the exp->add chain.
     The AV accumulator is drained PSUM->SBUF by the ACT right after the
     last AV matmul, freeing its 2 PSUM banks for the next pass (PSUM:
     2 Q-projection banks + 2x2 scores banks + 2 AV banks = 8 exactly).

Precision (identical math to the HW-verified baseline, rel_err ~2.0e-3 vs
the 2e-2 budget): x/W stream fp16, projections fp16 x fp16 -> fp32 PSUM,
Q/K kept fp16, exp output and V bf16 (DVE accumulates the denominator at
its 2x 16-bit rate), everything normalized in fp32.
"""

from contextlib import ExitStack

import numpy as np

import concourse.bass as bass
import concourse.tile as tile
from concourse import bacc, masks, mybir
from concourse.bass_utils import run_bass_kernel_spmd

F32 = mybir.dt.float32
BF16 = mybir.dt.bfloat16
F16 = mybir.dt.float16
AF = mybir.ActivationFunctionType

B = 2
L = 2048
D = 2048  # d_model (contraction dim of projections)
HD = 128  # head dim
NH = 4  # heads per core
QC = NH * HD  # q-channels per core = 512
DC = D // 128  # d-model chunks of 128 = 16
NLT = 4  # l tiles of 512
LKT = L // 128  # lk blocks of 128 = 16
N_CORES = 8
SCALE = 1.0 / float(np.sqrt(HD))


def build_kernel(ctx: ExitStack, tc: tile.TileContext, xT, wkvT, wqT, bq, bk, bv, outT):
    nc = tc.nc

    persist = ctx.enter_context(tc.tile_pool(name="persist", bufs=1))
    x_sb = [persist.tile([128, L], F16, tag=f"x{k}", name=f"x{k}") for k in range(DC)]
    wkv = [persist.tile([128, 2 * HD], F16, tag=f"wkv{k}", name=f"wkv{k}") for k in range(DC)]
    wq = [persist.tile([128, QC], F16, tag=f"wq{k}", name=f"wq{k}") for k in range(DC)]
    qT = [persist.tile([128, L], F16, tag=f"qT{h}", name=f"qT{h}") for h in range(NH)]  # [d, l]
    kT = persist.tile([128, L], F16, tag="kT", name="kT")  # [d, l]
    vT = [persist.tile([128, 512], BF16, tag=f"vT{t}", name=f"vT{t}") for t in range(NLT)]
    vN = persist.tile([128, L], BF16, tag="vN", name="vN")  # block j: [:, 128j:+128] = V[128j:+128, :]
    ones_bf = persist.tile([128, 128], BF16, tag="ones_bf", name="ones_bf")
    ident = persist.tile([128, 128], BF16, tag="ident", name="ident")
    bq_sb = persist.tile([128, NH], F32, tag="bq", name="bq")
    bk_sb = persist.tile([128, 1], F32, tag="bk", name="bk")
    bv_sb = persist.tile([128, 1], F32, tag="bv", name="bv")

    nc.vector.memset(ones_bf[:], 1.0)
    nc.scalar.dma_start(out=bq_sb[:], in_=bq)
    nc.scalar.dma_start(out=bk_sb[:], in_=bk)
    nc.scalar.dma_start(out=bv_sb[:], in_=bv)

    # ---------------- DMA plan ----------------
    # The ACT engine's DGE is too slow (~1.3us/issue) to put real traffic
    # on, and the SP queue issues ~0.3-0.6us/DMA, so x (8MB) is split by
    # urgency: [128,512] quarters for the l-columns the KV phase needs in
    # its first ~15us (lt=0,1), one [128,1024] half for lt=2,3.  Even d-
    # chunks issue from the gpsimd DGE, odd chunks + all weights from SP,
    # each queue ordered so pieces land just ahead of their first matmul.
    def x_dma(eng, k, c0, c1):
        eng.dma_start(out=x_sb[k][:, c0:c1], in_=xT[k * 128:(k + 1) * 128, c0:c1])

    evens = list(range(0, DC, 2))
    odds = list(range(1, DC, 2))
    # gpsimd: even chunks
    for lt in (0, 1):
        for k in evens:
            x_dma(nc.gpsimd, k, lt * 512, (lt + 1) * 512)
    for k in evens:
        x_dma(nc.gpsimd, k, 1024, 2048)
    # identity only needed by the V transposes (~40us in)
    masks.make_identity(nc, ident[:])
    # SP: wkv first (KV phase gates on them), interleaved with odd x chunks
    for k in range(3):
        nc.sync.dma_start(out=wkv[k][:], in_=wkvT[k * 128:(k + 1) * 128, :])
    oi = iter(odds)
    for k in range(3, DC):
        nc.sync.dma_start(out=wkv[k][:], in_=wkvT[k * 128:(k + 1) * 128, :])
        ko = next(oi, None)
        if ko is not None:
            x_dma(nc.sync, ko, 0, 512)
    for ko in oi:
        x_dma(nc.sync, ko, 0, 512)
    for k in odds:
        x_dma(nc.sync, k, 512, 1024)
    for k in odds:
        x_dma(nc.sync, k, 1024, 2048)
    for k in range(DC):
        nc.sync.dma_start(out=wq[k][:], in_=wqT[k * 128:(k + 1) * 128, :])

    # ---------------- PE warmup ----------------
    # The PE runs at 0.65/1.2 GHz until it has been continuously busy ~3us;
    # ramp it on throwaway matmuls while the first wkv/x DMAs land.
    with tc.tile_pool(name="warm", bufs=1, space="PSUM") as wp:
        pw = wp.tile([128, 128], F32, tag="warm", name="pw")
        for _ in range(30):
            nc.tensor.matmul(pw[:], lhsT=ones_bf[:], rhs=ones_bf[:], start=True, stop=True)

    # ---------------- KV phase: kT/vT for all of L ----------------
    with tc.tile_pool(name="kvp", bufs=2, space="PSUM") as kvp:
        for lt in range(NLT):
            ls = slice(lt * 512, (lt + 1) * 512)
            psk = kvp.tile([128, 512], F32, tag="psk", name="psk")
            psv = kvp.tile([128, 512], F32, tag="psv", name="psv")
            for k in range(DC):
                st = k == 0
                sp = k == DC - 1
                nc.tensor.matmul(psk[:], lhsT=wkv[k][:, 0:HD], rhs=x_sb[k][:, ls], start=st, stop=sp)
                nc.tensor.matmul(psv[:], lhsT=wkv[k][:, HD:], rhs=x_sb[k][:, ls], start=st, stop=sp)
            nc.scalar.activation(kT[:, ls], psk[:], AF.Identity, bias=bk_sb[:, 0:1])
            nc.scalar.activation(vT[lt][:], psv[:], AF.Identity, bias=bv_sb[:, 0:1])

    # ---------------- Q projections (as superstep filler) ----------------
    qp = ctx.enter_context(tc.tile_pool(name="qp", bufs=1, space="PSUM"))

    def q_sweep_items(t, pair):
        """Generator of emission thunks: 32 matmuls (heads 2*pair, 2*pair+1
        over 16 d-chunks) + 2 ACT drains. Lazily allocates its 2 PSUM tiles
        on first next()."""
        ls = slice(t * 512, (t + 1) * 512)
        ps = [qp.tile([128, 512], F32, tag=f"psq{j}", name=f"psq{j}") for j in (0, 1)]
        for k in range(DC):
            st = k == 0
            sp = k == DC - 1
            for j in (0, 1):
                h = 2 * pair + j
                yield lambda k=k, j=j, h=h, st=st, sp=sp: nc.tensor.matmul(
                    ps[j][:],
                    lhsT=wq[k][:, h * 128:(h + 1) * 128],
                    rhs=x_sb[k][:, ls],
                    start=st,
                    stop=sp,
                )
        for j in (0, 1):
            h = 2 * pair + j
            yield lambda j=j, h=h: nc.scalar.activation(
                qT[h][:, ls], ps[j][:], AF.Identity, bias=bq_sb[:, h:h + 1]
            )

    def run_all(gen):
        for th in gen:
            th()

    # Q l-tile 0 runs before the first superstep; the V transposes slot
    # between its two sweeps (their tpg PSUM bank is free again before the
    # attention pools open).
    run_all(q_sweep_items(0, 0))
    with tc.tile_pool(name="tpg", bufs=2, space="PSUM") as tpg:
        for j in range(LKT):
            pt = tpg.tile([128, 128], BF16, tag="tp", name="tp")
            nc.tensor.transpose(pt[:], vT[j // 4][:, (j % 4) * 128:(j % 4 + 1) * 128], ident[:])
            # copies on the DVE: the ACT is the attention-phase pacer and
            # must reach the first exp as soon as the Q drains are done
            nc.vector.tensor_copy(vN[:, j * 128:(j + 1) * 128], pt[:])
    run_all(q_sweep_items(0, 1))

    # ---------------- attention supersteps ----------------
    sps = ctx.enter_context(tc.tile_pool(name="sps", bufs=2, space="PSUM"))  # 2 x [128,1024] = 4 banks
    avp = ctx.enter_context(tc.tile_pool(name="avp", bufs=1, space="PSUM"))  # [128,1024] = 2 banks
    attp = ctx.enter_context(tc.tile_pool(name="att", bufs=6))
    accp = ctx.enter_context(tc.tile_pool(name="acc", bufs=2))
    avsbp = ctx.enter_context(tc.tile_pool(name="avsb", bufs=2))
    finp = ctx.enter_context(tc.tile_pool(name="fin", bufs=3))

    tail = {"pending": None}

    def emit_tail(info, split=2):
        """Softmax tail of a finished pass: partition-reduce+replicate the
        DVE-summed denominator with two 128-wide ones-matmuls (into a
        rotating scores PSUM slot), fast-reciprocal, normalize the
        ACT-drained AV sum, DMA out."""
        acc, av_sb, t, hp = info
        psR = sps.tile([128, 1024], F32, tag="sps", name="psR")
        for j in (0, 1):
            nc.tensor.matmul(
                psR[:, j * 512:(j + 1) * 512],
                lhsT=ones_bf[:],
                rhs=acc[:, j * 512:(j + 1) * 512],
                start=True,
                stop=True,
            )
        rinv = finp.tile([128, 1024], F32, tag="rinv", name="rinv")
        nc.vector.reciprocal_approx_fast(rinv[:], psR[:])
        ot = finp.tile([128, 1024], F32, tag="ot", name="ot")
        nc.vector.tensor_mul(ot[:], av_sb[:], rinv[:])
        n = 512 // split
        for j in (0, 1):
            h = 2 * hp + j
            for s in range(split):
                cs = slice(t * 512 + s * n, t * 512 + (s + 1) * n)
                nc.sync.dma_start(out=outT[h * 128:(h + 1) * 128, cs], in_=ot[:, j * 512 + s * n:j * 512 + (s + 1) * n])

    def emit_pass(t, hp, fill):
        qs = slice(t * 512, (t + 1) * 512)
        acc = accp.tile([128, 1024], BF16, tag="acc", name="acc")
        psA = avp.tile([128, 1024], F32, tag="av", name="av")
        at_of = {}

        def F(n=1):
            for _ in range(n):
                th = next(fill, None)
                if th is not None:
                    th()

        def sc(lk):
            ss = sps.tile([128, 1024], F32, tag="sps", name="sps")
            for j in (0, 1):
                nc.tensor.matmul(
                    ss[:, j * 512:(j + 1) * 512],
                    lhsT=kT[:, lk * 128:(lk + 1) * 128],
                    rhs=qT[2 * hp + j][:, qs],
                    start=True,
                    stop=True,
                )
            at = attp.tile([128, 1024], BF16, tag="att", name="att")
            nc.scalar.activation(at[:], ss[:], AF.Exp, scale=SCALE)
            # softmax denominator partials on the DVE (2x bf16 rate)
            if lk == 0:
                nc.vector.tensor_copy(acc[:], at[:])
            else:
                nc.vector.tensor_add(acc[:], acc[:], at[:])
            at_of[lk] = at

        def av(lk):
            for j in (0, 1):
                nc.tensor.matmul(
                    psA[:, j * 512:(j + 1) * 512],
                    lhsT=vN[:, lk * 128:(lk + 1) * 128],
                    rhs=at_of[lk][:, j * 512:(j + 1) * 512],
                    start=lk == 0,
                    stop=lk == LKT - 1,
                )

        # boundary: scores for blocks 0-1, fillers to cover the previous
        # pass's exp->add->psR chain, then that pass's tail
        sc(0)
        F(2)
        sc(1)
        F(4)
        if tail["pending"] is not None:
            emit_tail(tail["pending"])
            tail["pending"] = None
        F(4)
        # steady state: AV trails scores by 2 blocks, ~2 fillers per block
        for lk in range(2, LKT):
            sc(lk)
            F(1)
            av(lk - 2)
            if lk % 4 == 1:
                F(1)
        av(LKT - 2)
        F(1)
        av(LKT - 1)
        # drain the AV accumulator to SBUF on the ACT (frees its PSUM banks
        # for the next pass; the normalize multiply reads the SBUF copy)
        av_sb = avsbp.tile([128, 1024], F32, tag="avsb", name="avsb")
        nc.scalar.activation(av_sb[:], psA[:], AF.Identity)
        tail["pending"] = (acc, av_sb, t, hp)

    for t in range(NLT):
        if t + 1 < NLT:
            import itertools
            fill = itertools.chain(q_sweep_items(t + 1, 0), q_sweep_items(t + 1, 1))
        else:
            fill = iter(())
        emit_pass(t, 0, fill)
        emit_pass(t, 1, fill)
        run_all(fill)  # leftover fillers
    emit_tail(tail["pending"], split=4)
    tail["pending"] = None


_NC_CACHE = None


def build_nc():
    global _NC_CACHE
    if _NC_CACHE is not None:
        return _NC_CACHE
    nc = bacc.Bacc("TRN2", target_bir_lowering=False, debug=False)
    xT = nc.dram_tensor("xT", [D, L], F16, kind="ExternalInput").ap()
    wkvT = nc.dram_tensor("wkvT", [D, 2 * HD], F16, kind="ExternalInput").ap()
    wqT = nc.dram_tensor("wqT", [D, QC], F16, kind="ExternalInput").ap()
    bq = nc.dram_tensor("bq", [128, NH], F32, kind="ExternalInput").ap()
    bk = nc.dram_tensor("bk", [128, 1], F32, kind="ExternalInput").ap()
    bv = nc.dram_tensor("bv", [128, 1], F32, kind="ExternalInput").ap()
    outT = nc.dram_tensor("outT", [QC, L], F32, kind="ExternalOutput").ap()
    with tile.TileContext(nc) as tc, ExitStack() as ctx:
        build_kernel(ctx, tc, xT, wkvT, wqT, bq, bk, bv, outT)
    nc.compile()
    _NC_CACHE = nc
    return nc


def make_in_maps(x, Wq_w, Wq_b, Wk_w, Wk_b, Wv_w, Wv_b):
    """Host-side sharding/relayout. Returns one input map per core."""
    x = np.asarray(x, dtype=np.float32)
    Wq_w = np.asarray(Wq_w, dtype=np.float32)
    Wq_b = np.asarray(Wq_b, dtype=np.float32)
    Wk_w = np.asarray(Wk_w, dtype=np.float32)
    Wk_b = np.asarray(Wk_b, dtype=np.float32)
    Wv_w = np.asarray(Wv_w, dtype=np.float32)
    Wv_b = np.asarray(Wv_b, dtype=np.float32)

    xTs = [np.ascontiguousarray(x[b].T).astype(np.float16) for b in range(B)]
    wkvT = np.ascontiguousarray(
        np.concatenate([Wk_w.T, Wv_w.T], axis=1)
    ).astype(np.float16)  # [D, 256]
    bk = np.ascontiguousarray(Wk_b.reshape(128, 1))
    bv = np.ascontiguousarray(Wv_b.reshape(128, 1))
    in_maps = []
    for c in range(N_CORES):
        b, g = divmod(c, B * 2)  # b = c // 4, g = c % 4
        wqT_g = np.ascontiguousarray(Wq_w[g * QC:(g + 1) * QC, :].T).astype(np.float16)
        bq_g = np.ascontiguousarray(Wq_b[g * QC:(g + 1) * QC].reshape(NH, 128).T)
        in_maps.append(
            {
                "xT": xTs[b],
                "wkvT": wkvT,
                "wqT": wqT_g,
                "bq": bq_g,
                "bk": bk,
                "bv": bv,
            }
        )
    return in_maps


def assemble_output(results):
    out = np.empty((B, L, D), dtype=np.float32)
    for c in range(N_CORES):
        b, g = divmod(c, B * 2)
        out[b, :, g * QC:(g + 1) * QC] = results[c]["outT"].T
    return out


def kernel(**inputs) -> np.ndarray:
    nc = build_nc()
    in_maps = make_in_maps(**inputs)
    res = run_bass_kernel_spmd(nc, in_maps, core_ids=list(range(N_CORES)))
    return assemble_output(res.results)
